# revision 5
# baseline (speedup 1.0000x reference)
"""Transformer block (dense_transformer) on 8 TRN2 NeuronCores.

Strategy: pure data-parallel over batch (B=128 -> 16 items/core), weights
replicated. Per item, all linear layers run feature-major ([feat, T] with
feat on partitions) so matmul outputs land directly in the layout the next
matmul consumes; LayerNorm/softmax run token-major ([T, feat]) where their
reductions are free-axis reductions. PE transposes convert between the two.
Matmul operands use float32r (TF32-like, ~1 cyc/row at N>=256) — final
output error vs the fp32 reference is ~1e-4 absmax because matmul error only
perturbs the residual corrections.
"""

import numpy as np

import concourse.bass as bass
import concourse.mybir as mybir
from concourse.tile import TileContext
from concourse.vector_clock import ScopedClock

F32 = mybir.dt.float32
F32R = mybir.dt.float32r
AF = mybir.ActivationFunctionType
AX = mybir.AxisListType
ALU = mybir.AluOpType

B, T, C, H, D = 128, 256, 384, 6, 64
F = 4 * C
NCORES = 8
BL = B // NCORES
P = 128
TT = T // P    # 2 token tiles
CT = C // P    # 3 channel tiles
FT = F // P    # 12 ffn-hidden tiles
LN_EPS = 1e-5
CSCALE = float(C) ** -0.5
NEG = -1.0e9


class PatchedTileContext(TileContext):
    """Workaround for this container's walrus: BIR instructions may carry at
    most ONE attached sem wait. Hoist extras into standalone waits."""

    def _hoist_multi_waits(self):
        nc = self.nc
        assert self.sems is not None
        sem_by_num = {s.num: s for s in self.sems.allocated().values()}
        for func in nc.m.functions:
            for blk in func.blocks:
                insts = blk.instructions
                i = 0
                while i < len(insts):
                    inst = insts[i]
                    si = inst.sync_info
                    waits = list(si.on_wait) if (si and si.on_wait) else []
                    if len(waits) <= 1:
                        i += 1
                        continue
                    hoist = waits[1:]
                    for w in hoist:
                        if not (
                            w.sync_type == "semaphore"
                            and w.wait_mode == "sem-ge-imm"
                            and w.id in sem_by_num
                        ):
                            raise RuntimeError(
                                f"cannot hoist waits on {inst.name}: {waits}"
                            )
                    del si.on_wait[1:]
                    engine = nc.engines[inst.engine]
                    new_insts = []
                    for w in hoist:
                        wi = engine.wait_ge(sem_by_num[w.id], w.wait_value)
                        new_insts.append(wi.ins)
                    cur_list = nc.cur_bb.bb.instructions
                    for ni in new_insts:
                        cur_list.remove(ni)
                    insts[i:i] = new_insts
                    i += len(new_insts) + 1

    def _drain_and_barrier(self, tick_clock, wait_clock):
        nc = self.nc
        self._hoist_multi_waits()

        drain_inst = nc.sync.drain()
        wait_clock.add_sem_waits(
            drain_inst.ins, ScopedClock({None: tick_clock.global_clock})
        )
        waits = list(drain_inst.ins.sync_info.on_wait or [])
        if len(waits) > 1:
            drain_inst.ins.sync_info.on_wait.clear()
            assert self.sems is not None
            sem_by_num = {s.num: s for s in self.sems.allocated().values()}
            new_waits = []
            for w in waits:
                assert w.sync_type == "semaphore" and w.wait_mode == "sem-ge-imm", w
                new_waits.append(nc.sync.wait_ge(sem_by_num[w.id], w.wait_value))
            bb = nc.cur_bb.bb
            insts = bb.instructions
            names = [i.name for i in insts]
            di = names.index(drain_inst.ins.name)
            tail = insts[di + 1 : di + 1 + len(new_waits)]
            assert len(tail) == len(new_waits)
            insts[di : di + 1 + len(new_waits)] = tail + [drain_inst.ins]

        nc.all_engine_barrier()
        assert self.sems is not None
        popped = nc._tile_sem_poison_stack.pop()
        assert popped is self._sem_poison
        nc.clear_and_free_semaphores(list(self.sems.allocated().values()))
        nc.all_engine_barrier()


def ts(i, n=P):
    return slice(i * n, (i + 1) * n)


def build_nc():
    nc = bass.Bass()
    x_in = nc.dram_tensor("x", [BL, T, C], F32, kind="ExternalInput")
    wq_in = nc.dram_tensor("wqf", [C, C], F32, kind="ExternalInput")
    wk_in = nc.dram_tensor("wkf", [C, C], F32, kind="ExternalInput")
    wv_in = nc.dram_tensor("wvf", [C, C], F32, kind="ExternalInput")
    wp_in = nc.dram_tensor("wpf", [C, C], F32, kind="ExternalInput")
    w1_in = nc.dram_tensor("w1f", [C, F], F32, kind="ExternalInput")
    w2_in = nc.dram_tensor("w2f", [F, C], F32, kind="ExternalInput")
    gb_in = nc.dram_tensor("gb", [6, C], F32, kind="ExternalInput")  # g1,be1,g2,be2,bp,b2
    b1_in = nc.dram_tensor("b1v", [F], F32, kind="ExternalInput")
    id_in = nc.dram_tensor("ident", [P, P], F32, kind="ExternalInput")
    m_in = nc.dram_tensor("masks", [TT, P, T], F32, kind="ExternalInput")
    out_t = nc.dram_tensor("out", [BL, T, C], F32, kind="ExternalOutput")

    with PatchedTileContext(nc) as tc:
        with tc.tile_pool(name="consts", bufs=1) as consts:
            # ---- preload weights/constants (one-time) ----
            with tc.tile_pool(name="wstage", bufs=2) as wload:
                def load_f32r(ap_dram, kt, m, tag):
                    stage = wload.tile([P, kt, m], F32, tag="wstage")
                    nc.sync.dma_start(
                        stage[:], ap_dram.rearrange("(kt p) m -> p kt m", p=P)
                    )
                    wr = consts.tile([P, kt, m], F32R, tag=tag)
                    nc.vector.tensor_copy(wr[:], stage[:])
                    return wr

                wq_r = load_f32r(wq_in[:], CT, C, "wq")
                wk_r = load_f32r(wk_in[:], CT, C, "wk")
                wv_r = load_f32r(wv_in[:], CT, C, "wv")
                wp_r = load_f32r(wp_in[:], CT, C, "wp")
                w1_r = load_f32r(w1_in[:], CT, F, "w1")
                w2_r = load_f32r(w2_in[:], FT, C, "w2")

                id_f = consts.tile([P, P], F32, tag="idf")
                nc.sync.dma_start(id_f[:], id_in[:])
                id_r = consts.tile([P, P], F32R, tag="idr")
                nc.vector.tensor_copy(id_r[:], id_f[:])

                mstage = wload.tile([P, TT, T], F32, tag="wstage")
                nc.sync.dma_start(
                    mstage[:], m_in.rearrange("tt p t -> p tt t")
                )
                m_r = consts.tile([P, TT, T], F32R, tag="mr")
                nc.vector.tensor_copy(m_r[:], mstage[:])

                gb = consts.tile([P, 6, CT], F32, tag="gb")
                nc.sync.dma_start(gb[:], gb_in.rearrange("g (ct p) -> p g ct", p=P))
                b1c = consts.tile([P, FT], F32, tag="b1c")
                nc.sync.dma_start(b1c[:], b1_in.rearrange("(ft p) -> p ft", p=P))
                epsc = consts.tile([P, 1], F32, tag="eps")
                nc.gpsimd.memset(epsc[:], LN_EPS)

            g1c = gb[:, 0, :]
            be1c = gb[:, 1, :]
            g2c = gb[:, 2, :]
            be2c = gb[:, 3, :]
            bpc = gb[:, 4, :]
            b2c = gb[:, 5, :]

            with (
                tc.tile_pool(name="act", bufs=2) as actp,
                tc.tile_pool(name="fm", bufs=2) as fmp,
                tc.tile_pool(name="attn", bufs=3) as attnp,
                tc.tile_pool(name="stats", bufs=8) as stats,
                tc.tile_pool(name="ps", bufs=5, space="PSUM") as psp,
                tc.tile_pool(name="psbig", bufs=2, space="PSUM") as psbig,
                tc.tile_pool(name="psff", bufs=1, space="PSUM") as psff,
            ):
                for b in range(BL):
                    # ---- load x ----
                    x_t = actp.tile([P, TT, C], F32, tag="x")
                    nc.sync.dma_start(
                        x_t[:], x_in[b].rearrange("(tt p) c -> p tt c", p=P)
                    )

                    # ---- LN (shared impl) ----
                    def layernorm(src, gcol, becol, htag):
                        """src: [P, TT, C] token-major; returns h_ct [P, CT, T]
                        f32r feature-major (normalized, affine applied)."""
                        xn = actp.tile([P, TT, C], F32R, tag=htag + "xn")
                        for tt in range(TT):
                            ssum = stats.tile([P, 1], F32, tag="ssum")
                            nc.vector.reduce_sum(ssum[:], src[:, tt, :], axis=AX.X)
                            sq = actp.tile([P, C], F32, tag="sq")
                            ssq = stats.tile([P, 1], F32, tag="ssq")
                            nc.scalar.activation(
                                sq[:], src[:, tt, :], AF.Square, accum_out=ssq[:]
                            )
                            mu = stats.tile([P, 1], F32, tag="mu")
                            nc.scalar.mul(mu[:], ssum[:], 1.0 / C)
                            mu2 = stats.tile([P, 1], F32, tag="mu2")
                            nc.scalar.activation(mu2[:], mu[:], AF.Square)
                            var = stats.tile([P, 1], F32, tag="var")
                            # var = ssq/C - mu^2  (+eps folded into sqrt bias)
                            nc.scalar.mul(var[:], ssq[:], 1.0 / C)
                            nc.vector.tensor_tensor(
                                var[:], var[:], mu2[:], ALU.subtract
                            )
                            sd = stats.tile([P, 1], F32, tag="sd")
                            nc.scalar.activation(
                                sd[:], var[:], AF.Sqrt, bias=epsc[:]
                            )
                            rstd = stats.tile([P, 1], F32, tag="rstd")
                            nc.vector.reciprocal(rstd[:], sd[:])
                            nmur = stats.tile([P, 1], F32, tag="nmur")
                            nc.vector.tensor_tensor(
                                nmur[:], mu[:], rstd[:], ALU.mult
                            )
                            nc.scalar.mul(nmur[:], nmur[:], -1.0)
                            # xn = x*rstd - mu*rstd
                            nc.scalar.activation(
                                xn[:, tt, :], src[:, tt, :], AF.Identity,
                                bias=nmur[:], scale=rstd[:],
                            )
                        h_ct = fmp.tile([P, CT, T], F32R, tag=htag)
                        for ct in range(CT):
                            ps = psp.tile([P, T], F32R, tag="ps")
                            for tt in range(TT):
                                nc.tensor.transpose(
                                    ps[:, ts(tt)], xn[:, tt, ts(ct)], id_r[:]
                                )
                            nc.scalar.activation(
                                h_ct[:, ct, :], ps[:], AF.Identity,
                                bias=becol[:, ct : ct + 1],
                                scale=gcol[:, ct : ct + 1],
                            )
                        return h_ct

                    h_ct = layernorm(x_t, g1c, be1c, "h1")

                    # ---- QKV ----
                    qT = fmp.tile([P, CT, T], F32R, tag="qT")
                    kT = fmp.tile([P, CT, T], F32R, tag="kT")
                    for m in range(CT):
                        psq = psp.tile([P, T], F32, tag="ps")
                        psk = psp.tile([P, T], F32, tag="ps")
                        for k in range(CT):
                            nc.tensor.matmul(
                                psq[:], wq_r[:, k, ts(m)], h_ct[:, k, :],
                                start=(k == 0), stop=(k == CT - 1),
                            )
                            nc.tensor.matmul(
                                psk[:], wk_r[:, k, ts(m)], h_ct[:, k, :],
                                start=(k == 0), stop=(k == CT - 1),
                            )
                        nc.scalar.copy(qT[:, m, :], psq[:])
                        nc.vector.tensor_copy(kT[:, m, :], psk[:])
                    v_sb = fmp.tile([P, TT, C], F32R, tag="v")
                    for st in range(TT):
                        psv = psbig.tile([P, C], F32, tag="psb")
                        for k in range(CT):
                            nc.tensor.matmul(
                                psv[:], h_ct[:, k, ts(st)], wv_r[:, k, :],
                                start=(k == 0), stop=(k == CT - 1),
                            )
                        if st == 0:
                            nc.scalar.copy(v_sb[:, st, :], psv[:])
                        else:
                            nc.vector.tensor_copy(v_sb[:, st, :], psv[:])

                    # ---- attention ----
                    attnT = fmp.tile([P, CT, T], F32R, tag="attnT")
                    for h in range(H):
                        j, off = h // 2, (h % 2) * 64
                        w_n = attnp.tile([P, TT, T], F32R, tag="wn")
                        for tt in range(TT):
                            pss = psp.tile([P, T], F32, tag="ps")
                            nc.tensor.matmul(
                                pss[:],
                                qT[off : off + 64, j, ts(tt)],
                                kT[off : off + 64, j, :],
                                start=True, stop=False,
                            )
                            nc.tensor.matmul(
                                pss[:], id_r[:], m_r[:, tt, :],
                                start=False, stop=True,
                            )
                            we = attnp.tile([P, T], F32, tag="we")
                            rowsum = stats.tile([P, 1], F32, tag="rs")
                            nc.scalar.activation(
                                we[:], pss[:], AF.Exp,
                                scale=CSCALE, accum_out=rowsum[:],
                            )
                            rec = stats.tile([P, 1], F32, tag="rec")
                            nc.vector.reciprocal(rec[:], rowsum[:])
                            nc.vector.tensor_scalar_mul(
                                w_n[:, tt, :], we[:], rec[:]
                            )
                        wT = attnp.tile([P, TT, T], F32R, tag="wT")
                        for st in range(TT):
                            psw = psp.tile([P, T], F32R, tag="ps")
                            for tt in range(TT):
                                nc.tensor.transpose(
                                    psw[:, ts(tt)], w_n[:, tt, ts(st)], id_r[:]
                                )
                            if st == 0:
                                nc.scalar.copy(wT[:, st, :], psw[:])
                            else:
                                nc.vector.tensor_copy(wT[:, st, :], psw[:])
                        psa = psp.tile([64, T], F32, tag="ps")
                        for st in range(TT):
                            nc.tensor.matmul(
                                psa[:],
                                v_sb[:, st, h * 64 : (h + 1) * 64],
                                wT[:, st, :],
                                start=(st == 0), stop=(st == TT - 1),
                            )
                        if h % 2 == 0:
                            nc.scalar.copy(attnT[off : off + 64, j, :], psa[:])
                        else:
                            nc.vector.tensor_copy(
                                attnT[off : off + 64, j, :], psa[:]
                            )

                    # ---- proj + residual ----
                    saT = fmp.tile([P, CT, T], F32R, tag="saT")
                    for m in range(CT):
                        psj = psp.tile([P, T], F32, tag="ps")
                        for k in range(CT):
                            nc.tensor.matmul(
                                psj[:], wp_r[:, k, ts(m)], attnT[:, k, :],
                                start=(k == 0), stop=(k == CT - 1),
                            )
                        nc.scalar.activation(
                            saT[:, m, :], psj[:], AF.Identity,
                            bias=bpc[:, m : m + 1],
                        )
                    x1 = actp.tile([P, TT, C], F32, tag="x1")
                    for tt in range(TT):
                        psr = psbig.tile([P, C], F32R, tag="psb")
                        for ct in range(CT):
                            nc.tensor.transpose(
                                psr[:, ts(ct)], saT[:, ct, ts(tt)], id_r[:]
                            )
                        nc.vector.tensor_tensor(
                            x1[:, tt, :], psr[:], x_t[:, tt, :], ALU.add
                        )

                    # ---- LN2 + FFN ----
                    h2_ct = layernorm(x1, g2c, be2c, "h2")
                    z = fmp.tile([P, FT, T], F32R, tag="z")
                    for m in range(FT):
                        psf = psff.tile([P, T], F32, tag="psf")
                        for k in range(CT):
                            nc.tensor.matmul(
                                psf[:], w1_r[:, k, ts(m)], h2_ct[:, k, :],
                                start=(k == 0), stop=(k == CT - 1),
                            )
                        nc.scalar.activation(
                            z[:, m, :], psf[:], AF.Relu,
                            bias=b1c[:, m : m + 1],
                        )
                    yT = fmp.tile([P, CT, T], F32R, tag="yT")
                    for m in range(CT):
                        psy = psp.tile([P, T], F32, tag="ps")
                        for k in range(FT):
                            nc.tensor.matmul(
                                psy[:], w2_r[:, k, ts(m)], z[:, k, :],
                                start=(k == 0), stop=(k == FT - 1),
                            )
                        nc.scalar.activation(
                            yT[:, m, :], psy[:], AF.Identity,
                            bias=b2c[:, m : m + 1],
                        )

                    # ---- final residual + store ----
                    for tt in range(TT):
                        pso = psbig.tile([P, C], F32R, tag="psb")
                        for ct in range(CT):
                            nc.tensor.transpose(
                                pso[:, ts(ct)], yT[:, ct, ts(tt)], id_r[:]
                            )
                        o = actp.tile([P, C], F32, tag="o")
                        nc.vector.tensor_tensor(
                            o[:], pso[:], x1[:, tt, :], ALU.add
                        )
                        nc.sync.dma_start(out_t[b, ts(tt), :], o[:])
    return nc


_NC_CACHE = None


def _get_nc():
    global _NC_CACHE
    if _NC_CACHE is None:
        _NC_CACHE = build_nc()
    return _NC_CACHE


def _host_consts():
    ident = np.eye(P, dtype=np.float32)
    masks = np.zeros((TT, P, T), dtype=np.float32)
    for tt in range(TT):
        trow = np.arange(P) + tt * P
        scol = np.arange(T)
        masks[tt][scol[None, :] > trow[:, None]] = NEG
    return ident, masks


def kernel(x, Wq, Wk, Wv, Wp, bp, W1, b1, W2, b2, g1, be1, g2, be2):
    x = np.ascontiguousarray(np.asarray(x, np.float32))
    WqF = np.ascontiguousarray(
        np.asarray(Wq, np.float32).transpose(1, 0, 2).reshape(C, C)
    )
    WkF = np.ascontiguousarray(
        np.asarray(Wk, np.float32).transpose(1, 0, 2).reshape(C, C)
    )
    WvF = np.ascontiguousarray(
        np.asarray(Wv, np.float32).transpose(1, 0, 2).reshape(C, C)
    )
    WpF = np.ascontiguousarray(np.asarray(Wp, np.float32))
    W1F = np.ascontiguousarray(np.asarray(W1, np.float32))
    W2F = np.ascontiguousarray(np.asarray(W2, np.float32))
    gb = np.ascontiguousarray(
        np.stack([
            np.asarray(g1, np.float32), np.asarray(be1, np.float32),
            np.asarray(g2, np.float32), np.asarray(be2, np.float32),
            np.asarray(bp, np.float32), np.asarray(b2, np.float32),
        ])
    )
    b1v = np.ascontiguousarray(np.asarray(b1, np.float32))
    ident, masks = _host_consts()

    nc = _get_nc()
    shared = {
        "wqf": WqF, "wkf": WkF, "wvf": WvF, "wpf": WpF,
        "w1f": W1F, "w2f": W2F, "gb": gb, "b1v": b1v,
        "ident": ident, "masks": masks,
    }
    in_maps = []
    for c in range(NCORES):
        m = dict(shared)
        m["x"] = np.ascontiguousarray(x[c * BL : (c + 1) * BL])
        in_maps.append(m)

    from concourse.bass_utils import run_bass_kernel_spmd

    res = run_bass_kernel_spmd(nc, in_maps, list(range(NCORES)))
    out = np.concatenate([res.results[c]["out"] for c in range(NCORES)], axis=0)
    return out.astype(np.float32)
